# revision 70
# baseline (speedup 1.0000x reference)
"""DetectionLoss Trainium2 kernel — v3 (fp16 pair stage + spatial GT packing).

8-core data parallel, 4 images/core. Anchors are laid out spatially:
partition p = position inside an 8x16-anchor tile, free column g = tile id
(20x10 grid of tiles). Per (image, tile) the host packs only the GT boxes
that can possibly overlap the tile's decoded anchor boxes (exact f32 test,
conservative == reference-exact). Coordinates are shifted per-tile so the
fp16 pair stage keeps ~0.1px precision.

Device per image: fp16 decode, fp16 pairwise IoU-ratio r = inter/(a1+a2)
(monotone in IoU; thresholds 0.2 and 1/11 on r), segmented argmax via
tie-mask, PE-transpose gather of matched-GT quantities, fp16 log-softmax,
f32 smooth-L1, per-image partial sums [P, 6]. Host finishes the scalar
combine exactly like the reference.
"""
import numpy as np
import sys

sys.path.insert(0, "/opt/trn_rl_repo")

import concourse.bass as bass
import concourse.bacc as bacc
import concourse.mybir as mybir
from concourse import tile
from concourse.bass_utils import run_bass_kernel_spmd

F32 = mybir.dt.float32
F16 = mybir.dt.float16
ALU = mybir.AluOpType
ACT = mybir.ActivationFunctionType
AX = mybir.AxisListType

P = 128
G = 200
N = P * G
MGT = 50
C = 8
BPC = 4
NCORES = 8
NOUT = 6
FM = 160

# spatial tiling: tile = 8 anchor rows x 16 anchor cols; tile grid 20 x 10
TR_A, TC_A = 8, 16          # anchors per tile (rows, cols)
TGR, TGC = 20, 10           # tile grid
_p = np.arange(P)
_pr, _pc = _p // TC_A, _p % TC_A
_g = np.arange(G)
_tr, _tc = _g // TGC, _g % TGC
_R = _tr[None, :] * TR_A + _pr[:, None]     # [P,G] anchor row
_Cc = _tc[None, :] * TC_A + _pc[:, None]
PERM = (_R * FM + _Cc).reshape(-1)          # flat anchor idx for (p,g)
OFFX = ((_tc * TC_A + TC_A / 2.0) * 4.0).astype(np.float32)   # [G]
OFFY = ((_tr * TR_A + TR_A / 2.0) * 4.0).astype(np.float32)

RNEG = np.float32(1.0 / 11.0)   # r threshold for iou < 0.1
RPOS = np.float32(0.2)          # r threshold for iou >= 0.25


def _divisors(n):
    return [d for d in range(1, n + 1) if n % d == 0]


def _cfg_from_mp(mp_slot):
    """mp_slot: per-slot max packed-GT count (len 4). Returns static config."""
    ms = []
    for m in mp_slot:
        if m <= 0:
            ms.append(0)
        else:
            ms.append(int(2 * ((m + 1) // 2)))  # even, >= 2
    mmax = max(ms) if ms else 0
    nch = 2 if mmax <= 24 else 4
    gpc = G // nch
    ks = []
    for m in ms:
        if m == 0:
            ks.append(0)
            continue
        k = max(d for d in _divisors(gpc) if d * m <= 128)
        ks.append(k)
    return tuple(ms), tuple(ks), nch, gpc


def _bc(ap2d, m):
    """[p, gslice] anchor-plane slice -> [p, m, gslice] broadcast along m."""
    a = list(ap2d.ap)
    return bass.AP(ap2d.tensor, ap2d.offset, [a[0], [0, m]] + a[1:])


def build_program(cfg):
    ms, ks, nch, gpc = cfg
    mmax = max(ms)
    nc = bacc.Bacc(None, target_bir_lowering=False)

    big32_d = nc.dram_tensor("big32", [BPC, P, (C + 4) * G], F32,
                             kind="ExternalInput")
    anc16_d = nc.dram_tensor("anc16", [P, 6 * G], F16, kind="ExternalInput")
    anc32_d = nc.dram_tensor("anc32", [P, 6 * G], F32, kind="ExternalInput")
    iden_d = nc.dram_tensor("iden", [P, P], F16, kind="ExternalInput")
    gt_ds = {}
    for b in range(BPC):
        if ms[b] > 0:
            # per chunk: 5 planes [m, gpc] + rhs chunk appended on chunk 0
            gt_ds[b] = nc.dram_tensor(f"gt{b}", [nch, P, 5 * ms[b] * gpc], F16,
                                      kind="ExternalInput")
    rhs_ds = {}
    for b in range(BPC):
        if ms[b] > 0:
            nw = G // ks[b]
            rhs_ds[b] = nc.dram_tensor(f"rhs{b}", [P, nw * ks[b] * 5], F16,
                                       kind="ExternalInput")
    res_d = nc.dram_tensor("res", [P, BPC * NOUT], F32, kind="ExternalOutput")

    with tile.TileContext(nc) as tc:
        with (
            tc.tile_pool(name="const", bufs=1) as cpool,
            tc.tile_pool(name="img", bufs=3) as ipool,
            tc.tile_pool(name="work", bufs=2) as wpool,
            tc.tile_pool(name="ps", bufs=2, space="PSUM") as ppool,
            tc.tile_pool(name="pst", bufs=2, space="PSUM") as tpool,
        ):
            anc16 = cpool.tile([P, 6 * G], F16)
            anc32 = cpool.tile([P, 6 * G], F32)
            iden = cpool.tile([P, P], F16)
            res = cpool.tile([P, BPC * NOUT], F32)

            def a16(k):
                return anc16[:, k * G:(k + 1) * G]

            def a32(k):
                return anc32[:, k * G:(k + 1) * G]

            A_CXM, A_CYM, A_WH, A_HH, A_W, A_H = range(6)
            A_CX2, A_CY2, A_I2W, A_I2H, A_LW, A_LH = range(6)

            persist = {}
            s3all = cpool.tile([P, BPC * G], F16)
            for b in range(BPC):
                M = ms[b]
                kst = ks[b]
                big = ipool.tile([P, (C + 4) * G], F32, tag="big", name="big", bufs=2)
                nc.sync.dma_start(big[:], big32_d[b])
                if b == 0:
                    nc.sync.dma_start(anc16[:], anc16_d[:])
                    nc.sync.dma_start(iden[:], iden_d[:])
                    nc.sync.dma_start(anc32[:], anc32_d[:])

                def clsp(c):
                    return big[:, c * G:(c + 1) * G]

                def regp(c):
                    return big[:, (C + c) * G:(C + c + 1) * G]

                ot = res[:, b * NOUT:(b + 1) * NOUT]

                def dt16(tag):
                    return ipool.tile([P, G], F16, tag=tag, name=tag)

                # cls log-sum-exp issued early: act's exp/ln overlap the
                # DVE pair stage
                e16 = ipool.tile([P, C * G], F16, tag="e16", bufs=2, name="e16")
                nc.scalar.activation(e16[:], big[:, 0:C * G], ACT.Exp)
                if M > 0:
                    # ---- decode (fp16) ----
                    reg16 = ipool.tile([P, 4 * G], F16, tag="reg16",
                                       name="reg16")
                    nc.vector.tensor_copy(reg16[:], big[:, C * G:(C + 4) * G])

                    def r16(c):
                        return reg16[:, c * G:(c + 1) * G]

                    cx = dt16("cx"); cy = dt16("cy")
                    w = dt16("w"); h = dt16("h")
                    x1 = dt16("x1"); x2 = dt16("x2")
                    y1 = dt16("y1"); y2 = dt16("y2")
                    a1 = dt16("a1"); hw = dt16("hw")
                    nc.vector.tensor_tensor(cx[:], r16(0), a16(A_WH), ALU.mult)
                    nc.vector.tensor_tensor(cx[:], cx[:], a16(A_CXM), ALU.add)
                    nc.vector.tensor_tensor(cy[:], r16(1), a16(A_HH), ALU.mult)
                    nc.vector.tensor_tensor(cy[:], cy[:], a16(A_CYM), ALU.add)
                    ew2 = ipool.tile([P, 2 * G], F16, tag="ew2", bufs=2,
                                     name="ew2")
                    nc.scalar.activation(ew2[:], big[:, (C + 2) * G:
                                                      (C + 4) * G], ACT.Exp)
                    nc.vector.tensor_tensor(w[:], ew2[:, 0:G], a16(A_W),
                                            ALU.mult)
                    nc.vector.tensor_tensor(h[:], ew2[:, G:2 * G], a16(A_H),
                                            ALU.mult)
                    nc.vector.tensor_scalar(hw[:], w[:], 0.5, None, ALU.mult)
                    nc.vector.tensor_sub(x1[:], cx[:], hw[:])
                    nc.vector.tensor_add(x2[:], cx[:], hw[:])
                    nc.vector.tensor_scalar(hw[:], h[:], 0.5, None, ALU.mult)
                    nc.vector.tensor_sub(y1[:], cy[:], hw[:])
                    nc.vector.tensor_add(y2[:], cy[:], hw[:])
                    nc.vector.tensor_mul(a1[:], w[:], h[:])

                    rmxg = ipool.tile([P, G], F16, tag=f"rmx_{b}",
                                      name="rmxg", bufs=1)
                    pg5all = ipool.tile([P, G * 5], F32, tag="pg5all",
                                        name="pg5all")
                    rhs = ipool.tile([P, (G // kst) * kst * 5], F16, tag="rhs",
                                     name="rhs")
                    nc.sync.dma_start(rhs[:], rhs_ds[b][:])

                    for ch in range(nch):
                        g0 = ch * gpc
                        gsl = slice(g0, g0 + gpc)
                        gtc = wpool.tile([P, 5 * mmax * gpc], F16, tag="gtc",
                                         name="gtc", bufs=2 if nch > 1 else 1)
                        nc.sync.dma_start(gtc[:, 0:5 * M * gpc], gt_ds[b][ch])

                        def gplane(q):
                            sl = gtc[:, q * M * gpc:(q + 1) * M * gpc]
                            return sl.rearrange("p (m g) -> p m g", g=gpc)

                        def wt(tag):
                            t = wpool.tile([P, mmax * gpc], F16, tag=tag,
                                           name=tag)
                            sl = t[:, 0:M * gpc]
                            return t, sl.rearrange("p (m g) -> p m g", g=gpc)

                        ta, tav = wt("ta"); tb, tbv = wt("tb")
                        tiw, tiwv = wt("tiw"); tih, tihv = wt("tih")
                        tin, tinv = wt("tin"); tr_, trv = wt("tr")
                        # mask is written g-major (element (m,g) at g*M+m) so
                        # a transpose window is one contiguous free dim
                        tmk = wpool.tile([P, mmax * gpc], F16, tag="tmk",
                                         name="tmk")
                        tmkv = bass.AP(tmk[:].tensor, tmk[:].offset,
                                       [tmk[:].ap[0], [1, M], [M, gpc]])

                        nc.vector.tensor_tensor(tav, gplane(0), _bc(x1[:, gsl], M),
                                                ALU.max)
                        nc.vector.tensor_tensor(tbv, gplane(2), _bc(x2[:, gsl], M),
                                                ALU.min)
                        nc.vector.tensor_tensor(tiwv, tbv, tav, ALU.subtract)
                        nc.vector.tensor_tensor(tav, gplane(1), _bc(y1[:, gsl], M),
                                                ALU.max)
                        nc.vector.tensor_tensor(tbv, gplane(3), _bc(y2[:, gsl], M),
                                                ALU.min)
                        nc.vector.tensor_tensor(tihv, tbv, tav, ALU.subtract)
                        # single relu suffices: relu(iw)*ih has the right
                        # sign in every case (both-neg would otherwise flip +)
                        nc.vector.tensor_scalar(ta[:, 0:M * gpc],
                                                tiw[:, 0:M * gpc], 0.0, None,
                                                ALU.max)
                        nc.vector.tensor_tensor(tinv, tav, tihv, ALU.mult)
                        # s = a2 + a1  (reuse ta)
                        nc.vector.tensor_tensor(tav, gplane(4), _bc(a1[:, gsl], M),
                                                ALU.add)
                        with nc.allow_low_precision(reason="fp16 iou ratio"):
                            nc.vector.reciprocal(tb[:, 0:M * gpc],
                                                 ta[:, 0:M * gpc])
                        nc.vector.tensor_tensor(trv, tinv, tbv, ALU.mult)
                        # segmented max over m: pairwise halving then reduce
                        h1, h1v = wt("h1")
                        mh = M // 2
                        nc.vector.tensor_tensor(
                            h1[:, 0:mh * gpc].rearrange("p (m g) -> p m g",
                                                        g=gpc),
                            trv[:, 0:mh, :], trv[:, mh:M, :], ALU.max)
                        hT = bass.AP(h1[:].tensor, h1[:].offset,
                                     [h1[:].ap[0], [1, gpc], [gpc, mh]])
                        nc.vector.tensor_reduce(
                            rmxg[:, gsl].rearrange("p (u g) -> p u g", u=1),
                            hT, AX.X, ALU.max)
                        nc.vector.tensor_tensor(tmkv, trv, _bc(rmxg[:, gsl], M),
                                                ALU.is_equal)

                        # ---- gather: transpose windows + matmul ----
                        pg5ps = ppool.tile([P, gpc * 5], F32, tag="pg5ps",
                                           name="pg5ps")
                        nwc = gpc // kst
                        WB = 8
                        for w0 in range(0, nwc, WB):
                            wn = min(WB, nwc - w0)
                            psT = tpool.tile([P, WB * P], F16, tag="psT",
                                             name="psT")
                            sT = wpool.tile([P, WB * P], F16, tag="sT",
                                            name="sT")
                            for dw in range(wn):
                                wi = w0 + dw
                                mseg = bass.AP(
                                    tmk[:].tensor,
                                    tmk[:].offset + wi * kst * M,
                                    [tmk[:].ap[0], [1, kst * M]])
                                nc.tensor.transpose(
                                    psT[0:M * kst, dw * P:(dw + 1) * P],
                                    mseg, iden[:])
                            nc.scalar.activation(sT[0:M * kst, 0:wn * P],
                                                 psT[0:M * kst, 0:wn * P],
                                                 ACT.Copy)
                            for dw in range(wn):
                                wi = w0 + dw
                                wglob = ch * nwc + wi
                                nc.tensor.matmul(
                                    pg5ps[:, wi * kst * 5:(wi + 1) * kst * 5],
                                    sT[0:M * kst, dw * P:(dw + 1) * P],
                                    rhs[0:M * kst,
                                        wglob * kst * 5:(wglob + 1) * kst * 5],
                                    start=True, stop=True)
                        dstp = pg5all[:, ch * gpc * 5:(ch + 1) * gpc * 5]
                        nc.scalar.activation(dstp, pg5ps[:], ACT.Copy)

                # ---- cls sum tree -> per-image s3; ln deferred to tail ----
                s1 = ipool.tile([P, 4 * G], F16, tag="s1", bufs=2, name="s1")
                nc.gpsimd.tensor_tensor(s1[:], e16[:, 0:4 * G],
                                        e16[:, 4 * G:8 * G], ALU.add)
                s2 = ipool.tile([P, 2 * G], F16, tag="s2", bufs=2, name="s2")
                nc.gpsimd.tensor_tensor(s2[:], s1[:, 0:2 * G],
                                        s1[:, 2 * G:4 * G], ALU.add)
                nc.gpsimd.tensor_tensor(s3all[:, b * G:(b + 1) * G],
                                        s2[:, 0:G], s2[:, G:2 * G], ALU.add)
                c0b = ipool.tile([P, G], F16, tag=f"c0_{b}", name="c0b",
                                 bufs=1)
                nc.vector.tensor_copy(c0b[:], clsp(0))
                persist[b] = dict(c0=c0b)

                if M > 0:
                    persist[b]["rmx"] = rmxg

                    def gq(q):
                        t = pg5all[:]
                        return bass.AP(t.tensor, t.offset + q,
                                       [t.ap[0], [5, G]])

                    def gq2(q):
                        t = pg5all[:]
                        return bass.AP(t.tensor, t.offset + q,
                                       [t.ap[0], [1, 2], [5, G]])

                    # xt = cls[label]; label plane strided from pg5
                    xt8 = ipool.tile([P, C * G], F16, tag="xt8", bufs=2, name="xt8")
                    for ci in range(C):
                        nc.vector.scalar_tensor_tensor(
                            xt8[:, ci * G:(ci + 1) * G], gq(4), float(ci),
                            clsp(ci), ALU.is_equal, ALU.mult)
                    xt4 = ipool.tile([P, 4 * G], F16, tag="xt4", bufs=2, name="xt4")
                    nc.gpsimd.tensor_tensor(xt4[:], xt8[:, 0:4 * G],
                                            xt8[:, 4 * G:8 * G], ALU.add)
                    xt2 = ipool.tile([P, 2 * G], F16, tag="xt2", bufs=2, name="xt2")
                    nc.gpsimd.tensor_tensor(xt2[:], xt4[:, 0:2 * G],
                                            xt4[:, 2 * G:4 * G], ALU.add)
                    xtb = ipool.tile([P, G], F16, tag=f"xt_{b}", name="xtb",
                                     bufs=1)
                    nc.gpsimd.tensor_tensor(xtb[:], xt2[:, 0:G],
                                            xt2[:, G:2 * G], ALU.add)
                    persist[b]["xt"] = xtb

                    # ---- regression smooth-L1 (batched over 4 comps) ----
                    u2 = ipool.tile([P, 2 * G], F32, tag="u2", bufs=2, name="u2")
                    u2v = u2[:].rearrange("p (q g) -> p q g", g=G)
                    d4 = ipool.tile([P, 4 * G], F32, tag="d4", bufs=2, name="d4")
                    nc.vector.tensor_tensor(u2v, gq2(0),
                                            anc32[:, 0:2 * G].rearrange(
                                                "p (q g) -> p q g", g=G),
                                            ALU.subtract)
                    nc.vector.tensor_tensor(u2[:], u2[:],
                                            anc32[:, 2 * G:4 * G], ALU.mult)
                    nc.vector.tensor_tensor(d4[:, 0:2 * G],
                                            big[:, C * G:(C + 2) * G], u2[:],
                                            ALU.subtract)
                    nc.vector.tensor_tensor(u2v, gq2(2),
                                            anc32[:, 4 * G:6 * G].rearrange(
                                                "p (q g) -> p q g", g=G),
                                            ALU.subtract)
                    nc.vector.tensor_tensor(d4[:, 2 * G:4 * G],
                                            big[:, (C + 2) * G:(C + 4) * G],
                                            u2[:], ALU.subtract)
                    ab4 = ipool.tile([P, 4 * G], F16, tag="ab4", bufs=2, name="ab4")
                    nc.scalar.activation(ab4[:], d4[:], ACT.Abs)
                    z4 = ipool.tile([P, 4 * G], F16, tag="z4", bufs=2, name="z4")
                    nc.vector.tensor_scalar(z4[:], ab4[:], 1.0, None, ALU.min)
                    zh4 = ipool.tile([P, 4 * G], F16, tag="zh4", bufs=2, name="zh4")
                    nc.vector.tensor_scalar(zh4[:], z4[:], 0.5, None, ALU.mult)
                    nc.vector.tensor_tensor(zh4[:], zh4[:], ab4[:],
                                            ALU.subtract)
                    nc.vector.tensor_tensor(z4[:], z4[:], zh4[:], ALU.mult)
                    ns2 = ipool.tile([P, 2 * G], F16, tag="ns2", bufs=2, name="ns2")
                    nc.gpsimd.tensor_tensor(ns2[:], z4[:, 0:2 * G],
                                            z4[:, 2 * G:4 * G], ALU.add)
                    nslb = ipool.tile([P, G], F16, tag=f"nsl_{b}", name="nslb",
                                      bufs=1)
                    nc.gpsimd.tensor_tensor(nslb[:], ns2[:, 0:G],
                                            ns2[:, G:2 * G], ALU.add)
                    persist[b]["nsl"] = nslb

            # ---- tail: one ln for all images + partial sums ----
            lseall = cpool.tile([P, BPC * G], F32)
            nc.scalar.activation(lseall[:], s3all[:], ACT.Ln)
            for b in range(BPC):
                pb = persist[b]
                ot = res[:, b * NOUT:(b + 1) * NOUT]
                lse = lseall[:, b * G:(b + 1) * G]
                bgt = ipool.tile([P, G], F32, tag="bgt", name="bgt")
                nc.vector.scalar_tensor_tensor(bgt[:], pb["c0"][:], -1.0,
                                               lse, ALU.mult, ALU.add,
                                               accum_out=ot[:, 2:3])
                scr16 = ipool.tile([P, G], F16, tag="scr16", name="scr16")
                scr32 = ipool.tile([P, G], F32, tag="scr32", name="scr32")
                if ms[b] > 0:
                    posf = ipool.tile([P, G], F16, tag="posf", name="posf")
                    negf = ipool.tile([P, G], F16, tag="negf", name="negf")
                    nc.vector.tensor_scalar(posf[:], pb["rmx"][:], float(RPOS),
                                            0.0, ALU.is_ge, ALU.add,
                                            accum_out=ot[:, 0:1])
                    nc.vector.tensor_scalar(negf[:], pb["rmx"][:], float(RNEG),
                                            0.0, ALU.is_lt, ALU.add,
                                            accum_out=ot[:, 1:2])
                    ct = ipool.tile([P, G], F32, tag="ct", name="ct")
                    nc.vector.scalar_tensor_tensor(ct[:], pb["xt"][:], -1.0,
                                                   lse, ALU.mult, ALU.add)
                    nc.vector.scalar_tensor_tensor(scr32[:], ct[:], 1.0,
                                                   posf[:], ALU.mult, ALU.mult,
                                                   accum_out=ot[:, 3:4])
                    nc.vector.scalar_tensor_tensor(scr32[:], bgt[:], 1.0,
                                                   negf[:], ALU.mult, ALU.mult,
                                                   accum_out=ot[:, 4:5])
                    nc.vector.scalar_tensor_tensor(scr16[:], pb["nsl"][:], 1.0,
                                                   posf[:], ALU.mult, ALU.mult,
                                                   accum_out=ot[:, 5:6])
                else:
                    nc.vector.memset(ot[:, 0:2], 0.0)
                    nc.vector.memset(ot[:, 3:6], 0.0)

            nc.sync.dma_start(res_d[:], res[:])
    nc.compile()
    return nc


_NC_CACHE = {}


def _get_nc(cfg):
    if cfg not in _NC_CACHE:
        _NC_CACHE[cfg] = build_program(cfg)
    return _NC_CACHE[cfg]


def _tile_bounds(reg_output):
    """Per-image per-tile decoded-anchor bounding boxes. [B, G, 4] f32."""
    B = reg_output.shape[0]
    r = reg_output.reshape(B, 4, FM, FM).astype(np.float32)
    cgrid = (np.arange(FM, dtype=np.float32) + 0.5) * 4.0
    w_dec = 32.0 * np.exp(r[:, 2])
    h_dec = 32.0 * np.exp(r[:, 3])
    cx_dec = cgrid[None, None, :] + (2.0 * r[:, 0] - 1.0) * 8.0
    cy_dec = cgrid[None, :, None] + (2.0 * r[:, 1] - 1.0) * 8.0
    x1 = cx_dec - w_dec / 2; x2 = cx_dec + w_dec / 2
    y1 = cy_dec - h_dec / 2; y2 = cy_dec + h_dec / 2

    def tb(a, op):
        t = a.reshape(B, TGR, TR_A, TGC, TC_A)
        return t.min((2, 4)) if op == 'min' else t.max((2, 4))

    tx1 = tb(x1, 'min').reshape(B, G); tx2 = tb(x2, 'max').reshape(B, G)
    ty1 = tb(y1, 'min').reshape(B, G); ty2 = tb(y2, 'max').reshape(B, G)
    return np.stack([tx1, ty1, tx2, ty2], axis=-1)


PAD_C = np.float32(2.0e4)
PAD_A = np.float32(3.0e4)
SC = np.float32(0.25)   # coordinate scale: keeps 1/(a1+a2) in fp16 normal range


def prep_inputs(cls_output, reg_output, anchors, gt_boxes, gt_labels,
                num_boxes):
    B = cls_output.shape[0]
    cls_output = np.asarray(cls_output, np.float32)
    reg_output = np.asarray(reg_output, np.float32)
    anchors = np.asarray(anchors, np.float32)
    gt_boxes = np.asarray(gt_boxes, np.float32)
    gt_labels = np.asarray(gt_labels)
    num_boxes = np.asarray(num_boxes)

    aw = anchors[:, 2] - anchors[:, 0]
    ah = anchors[:, 3] - anchors[:, 1]
    acx = anchors[:, 0] + 0.5 * aw
    acy = anchors[:, 1] + 0.5 * ah

    def pg(v):
        return v[PERM].reshape(P, G)

    a16 = np.stack([
        (pg(acx - aw / 4.0) - OFFX[None, :]) * SC,
        (pg(acy - ah / 4.0) - OFFY[None, :]) * SC,
        pg(aw / 2.0) * SC, pg(ah / 2.0) * SC, pg(aw) * SC, pg(ah) * SC,
    ], axis=1).astype(np.float16).reshape(P, 6 * G)
    # CX2 = acx - aw/4 (folds the +0.5 of the reg target); shifted+scaled
    a32p = np.stack([
        (pg(acx - aw / 4.0) - OFFX[None, :]) * SC,
        (pg(acy - ah / 4.0) - OFFY[None, :]) * SC,
        pg(2.0 / aw) / SC, pg(2.0 / ah) / SC, pg(np.log(aw)), pg(np.log(ah)),
    ], axis=1).astype(np.float32).reshape(P, 6 * G)

    cls_h = cls_output.reshape(B, C, N)[:, :, PERM].reshape(B, C, P, G)
    reg_h = reg_output.reshape(B, 4, N)[:, :, PERM].reshape(B, 4, P, G)
    big = np.concatenate([cls_h, reg_h], axis=1) \
        .transpose(0, 2, 1, 3).reshape(B, P, (C + 4) * G).astype(np.float32)

    # --- relevance packing ---
    tbx = _tile_bounds(reg_output)              # [B, G, 4]
    valid = np.arange(MGT)[None, :] < num_boxes[:, None]
    gx1 = gt_boxes[..., 0]; gy1 = gt_boxes[..., 1]
    gx2 = gt_boxes[..., 2]; gy2 = gt_boxes[..., 3]
    ox = (np.minimum(tbx[:, :, None, 2], gx2[:, None, :]) -
          np.maximum(tbx[:, :, None, 0], gx1[:, None, :])) > 0
    oy = (np.minimum(tbx[:, :, None, 3], gy2[:, None, :]) -
          np.maximum(tbx[:, :, None, 1], gy1[:, None, :])) > 0
    rel = ox & oy & valid[:, None, :]           # [B, G, M]
    cnt = rel.sum(-1)                           # [B, G]
    mp = cnt.max(-1)                            # [B]

    order = np.argsort(-mp, kind='stable')      # images sorted by Mp desc
    mp_slot = [int(mp[order[s * NCORES:(s + 1) * NCORES]].max())
               for s in range(BPC)]
    cfg = _cfg_from_mp(mp_slot)
    ms, ks, nch, gpc = cfg

    # packed GT per (image, tile): local indices via stable argsort
    gorder = np.argsort(~rel, axis=-1, kind='stable')   # [B, G, M]

    gw = gx2 - gx1; gh = gy2 - gy1
    gcx = gx1 + 0.5 * gw; gcy = gy1 + 0.5 * gh
    lgw = np.log(np.maximum(gw, 1e-6)); lgh = np.log(np.maximum(gh, 1e-6))
    labf = gt_labels.astype(np.float32)

    in_maps = [dict() for _ in range(NCORES)]
    core_imgs = [[int(order[s * NCORES + c]) for s in range(BPC)]
                 for c in range(NCORES)]
    for c in range(NCORES):
        im = in_maps[c]
        im["big32"] = np.ascontiguousarray(big[core_imgs[c]])
        im["anc16"] = a16
        im["anc32"] = a32p
        im["iden"] = np.eye(P, dtype=np.float16)
        for s in range(BPC):
            M = ms[s]
            if M == 0:
                continue
            bi_ = core_imgs[c][s]
            idx = gorder[bi_, :, :M]                    # [G, M] gt indices
            sel = np.arange(M)[None, :] < cnt[bi_][:, None]   # [G, M] valid
            def take(v, shift=None):
                t = v[bi_][idx]                         # [G, M]
                if shift is not None:
                    t = (t - shift[:, None]) * SC
                return np.where(sel, t, PAD_C).astype(np.float16)
            p_x1 = take(gx1, OFFX); p_y1 = take(gy1, OFFY)
            p_x2 = take(gx2, OFFX); p_y2 = take(gy2, OFFY)
            p_a2 = np.where(sel,
                            ((gx2 - gx1) * (gy2 - gy1))[bi_][idx] * (SC * SC),
                            PAD_A).astype(np.float16)
            # device layout per chunk: planes [5][M][gpc]
            planes = np.stack([p_x1, p_y1, p_x2, p_y2, p_a2], axis=0)  # [5,G,M]
            planes = planes.transpose(0, 2, 1).reshape(5, M, nch, gpc) \
                .transpose(2, 0, 1, 3).reshape(nch, 5 * M * gpc)
            gt_full = np.broadcast_to(planes[:, None, :],
                                      (nch, P, 5 * M * gpc))
            im[f"gt{s}"] = np.ascontiguousarray(gt_full)

            # rhs: [M*k, NW*k*5] block-diagonal gather weights
            kq = ks[s]
            nw = G // kq
            qv = np.stack([
                np.where(sel, (gcx[bi_][idx] - OFFX[:, None]) * SC, 0.0),
                np.where(sel, (gcy[bi_][idx] - OFFY[:, None]) * SC, 0.0),
                np.where(sel, lgw[bi_][idx], 0.0),
                np.where(sel, lgh[bi_][idx], 0.0),
                np.where(sel, labf[bi_][idx], 0.0),
            ], axis=-1).astype(np.float16)              # [G, M, 5]
            rhs = np.zeros((M * kq, nw, kq * 5), np.float16)
            # transpose emits row j = dg*M + m (mask stored g-major)
            rv = rhs.reshape(kq, M, nw, kq, 5)
            for dg in range(kq):
                # groups dg, dg+kq, ... -> windows 0..nw-1
                rv[dg, :, :, dg, :] = qv[dg::kq].transpose(1, 0, 2)
            rhs_full = np.zeros((P, nw * kq * 5), np.float16)
            rhs_full[:M * kq] = rhs.reshape(M * kq, nw * kq * 5)
            im[f"rhs{s}"] = rhs_full
    return cfg, in_maps, core_imgs, num_boxes


def finish(res_all, num_boxes):
    s = res_all.sum(axis=1).astype(np.float32)          # [B, NOUT]
    npos, nneg, ce_bg_sum, ce_tgt_pos, ce_bg_neg, neg_sl = \
        (s[:, i] for i in range(NOUT))
    sl_pos = -neg_sl
    has = num_boxes > 0
    cls_pos = np.where(npos > 0, ce_tgt_pos / np.maximum(npos, 1.0), 0.0)
    cls_neg = np.where(nneg > 0, ce_bg_neg / np.maximum(nneg, 1.0), 0.0)
    cls_losses = np.where(has, cls_pos + cls_neg, ce_bg_sum / np.float32(N))
    reg_losses = np.where(npos > 0, sl_pos / np.maximum(npos * 4.0, 1.0), 0.0)
    total_pos = npos.sum(dtype=np.float32)
    cls_final = np.float32(cls_losses.astype(np.float32).mean())
    reg_final = np.float32(reg_losses.astype(np.float32).sum()
                           / max(total_pos, 1.0))
    total = np.float32(cls_final + reg_final)
    return total, cls_final, reg_final, np.float32(total_pos)


def kernel(cls_output, reg_output, anchors, gt_boxes, gt_labels, num_boxes):
    cfg, in_maps, core_imgs, num_boxes = prep_inputs(
        cls_output, reg_output, anchors, gt_boxes, gt_labels, num_boxes)
    nc = _get_nc(cfg)
    out = run_bass_kernel_spmd(nc, in_maps, list(range(NCORES)))
    B = cls_output.shape[0]
    res_all = np.zeros((B, P, NOUT), np.float32)
    for c in range(NCORES):
        r = np.asarray(out.results[c]["res"]).reshape(P, BPC, NOUT)
        for s in range(BPC):
            res_all[core_imgs[c][s]] = r[:, s, :]
    return finish(res_all, np.asarray(num_boxes))
